# revision 25
# baseline (speedup 1.0000x reference)
"""Confusion-matrix (joint histogram) kernel for Trainium2.

Math: out[b, i, j] = #{pixels p in batch b : yp[b,p] == i and y[b,p] == j}
for i, j in [0, 21). Inputs yp, y are [8, 2048, 2048] int32, values in [0, 21).

Per NeuronCore (core c processes batch c), mixed-basis encoding:
each class-slot column s of a 128-wide block holds f_s(v) where
  slots 0..NS-1   : sign-range masks  S_{s+1}(v) = sign(v - s - 0.5)  (ACT,
                    reads int32 directly, one pass per slot)
  slots NS..19    : one-hot masks     [v == s]                        (DVE
                    is_equal at 4x mode; optionally a few on GpSimd)
  slot 20         : constant 1.0  (memset once per plane buffer)
  cols 126..127   : padding so weight blocks are 128 wide (enables the
                    compiler's fast-weight-load path); contents garbage,
                    confined to out rows 126/127 which the host ignores.

TensorE accumulates X' = F C F^T over all pixel blocks (G=6 pixel-column
groups per 128-wide block, moving operand 126 cols), where C is the true
confusion matrix and F the slot-encoding matrix. Host decodes
C = F^-1 X F^-T exactly in float64 (all X entries are integers < 2^24,
sign masks are +-1 so PSUM fp32 accumulation is exact).
"""

import numpy as np

C = 21                  # classes
G = 6                   # pixel-column groups per block
M = G * C               # 126 used columns
BLK = 128               # padded block width (weights FWL wants 128)
P = 128                 # partitions
FP = 840                # pixel-chunk columns per tensor (divisible by 6)
NS = 4                  # sign-mask slots computed on ACT (slots 0..NS-1)
N_GP = 0                # trailing delta slots computed on GpSimd
ONES_SLOT = C - 1       # slot 20: constant ones (marginals)
SENTINEL = 64           # int32 pad value outside [0, 21)
N_FREE = 32768          # 2048*2048 / 128

_CACHE = {}

RAMP = 216              # small first/last chunk: shrinks PE ramp-in + drain


def _schedule(n_free, fp=FP):
    """Chunk widths (all %6==0) plus trailing real-column remainder for a
    small final chunk (padded to %6 with sentinels by the builder)."""
    rem = 210 + (n_free % 6)
    if n_free <= rem + RAMP + fp:
        rem = n_free % 6
        widths = [n_free - rem] if n_free - rem else []
        return widths, rem
    body = n_free - rem
    k = (body - RAMP) // fp
    leftover = body - RAMP - k * fp
    widths = [RAMP] + [fp] * k
    if leftover:
        widths.append(leftover)
    return widths, rem


def _f_matrix(ns=NS):
    """F[s, v] = f_s(v): slot-encoding matrix, and its value at SENTINEL."""
    F = np.zeros((C, C), dtype=np.float64)
    v = np.arange(C)
    for s in range(ns):
        F[s] = np.where(v >= s + 1, 1.0, -1.0)
    for s in range(ns, C - 1):
        F[s, s] = 1.0
    F[C - 1] = 1.0
    u = np.zeros(C, dtype=np.float64)
    u[:ns] = 1.0          # sign(SENTINEL - s - 0.5) = +1
    u[C - 1] = 1.0        # ones
    assert abs(np.linalg.det(F)) > 0.5
    return F, u


def _build(n_free=N_FREE, fp=FP, ns=NS, n_gp=N_GP):
    import concourse.bacc as bacc
    import concourse.mybir as mybir
    import concourse.tile as tile

    nc = bacc.Bacc(
        "TRN2",
        target_bir_lowering=False,
        debug=False,
        enable_asserts=False,
        num_devices=8,
    )
    yp = nc.dram_tensor("yp", [P, n_free], mybir.dt.int32, kind="ExternalInput").ap()
    y = nc.dram_tensor("y", [P, n_free], mybir.dt.int32, kind="ExternalInput").ap()
    out = nc.dram_tensor(
        "out", [M, 2 * M], mybir.dt.float32, kind="ExternalOutput"
    ).ap()

    sched, rem = _schedule(n_free, fp)
    tail_pad = -rem % G if rem else 0
    tail_w = rem + tail_pad                      # final (padded) chunk width
    if tail_w:
        sched = sched + [tail_w]
    # Basis parity per chunk: every 5th full-size chunk runs with one extra
    # ACT sign slot (ns+1) and one fewer DVE slot, accumulating into a second
    # PSUM tile with its own basis — harvests ACT's idle slack.
    parity = [
        1 if (w == fp and wi % 5 == 0 and wi > 0) else 0
        for wi, w in enumerate(sched)
    ]
    mms_total = [
        sum(w // G for w, p in zip(sched, parity) if p == i) for i in (0, 1)
    ]
    nblk_max = 2 * fp // G                       # blocks in a full chunk (both halves)

    bf16 = mybir.dt.bfloat16
    f32 = mybir.dt.float32
    i32 = mybir.dt.int32
    Copy = mybir.ActivationFunctionType.Copy
    Sign = mybir.ActivationFunctionType.Sign
    is_equal = mybir.AluOpType.is_equal

    with tile.TileContext(nc) as tc:
        with (
            tc.tile_pool(name="psum", bufs=1, space="PSUM") as psum_pool,
            tc.tile_pool(name="cat", bufs=3) as cat_pool,
            tc.tile_pool(name="singles", bufs=1) as singles,
        ):
            accs = [
                psum_pool.tile([P, M], f32, tag=f"acc{i}", name=f"acc{i}")
                for i in (0, 1)
            ]
            # Per-slot bias vectors for the ACT sign masks (bias must be a
            # [128, 1] SBUF AP for non-Copy activations).
            sbias = singles.tile([P, ns + 1], f32)
            for s in range(ns + 1):
                nc.vector.memset(sbias[:, s : s + 1], -(s + 0.5))
            # Dummy activation: triggers the lazy ACT_TABLE_LOAD (~2.7us)
            # during the first DMA wait instead of on chunk 0's conversion.
            warm = singles.tile([P, 1], bf16)
            nc.scalar.activation(warm[:], sbias[:, 0:1], Sign, bias=sbias[:, 0:1])
            # Two persistent plane buffers (manual double-buffer) so the
            # ones-columns survive across chunks after a single memset.
            planes_bufs = []
            for bi in range(2):
                pb = singles.tile(
                    [P, nblk_max * BLK], bf16, tag=f"planes{bi}", name=f"planes{bi}"
                )
                pv = pb.rearrange("p (b f) -> p b f", f=BLK)
                nc.vector.memset(
                    pv[:, :, ONES_SLOT * G : (ONES_SLOT + 1) * G], 1.0
                )
                planes_bufs.append(pb)
            # Third (small) buffer for the final chunk: frees it from the
            # 2-buffer WAR wait on PE at the end of the stream.
            planes_tail = None
            if 0 < tail_w <= RAMP:
                planes_tail = singles.tile(
                    [P, (2 * tail_w // G) * BLK], bf16, tag="planes_t"
                )
                ptv = planes_tail.rearrange("p (b f) -> p b f", f=BLK)
                nc.vector.memset(
                    ptv[:, :, ONES_SLOT * G : (ONES_SLOT + 1) * G], 1.0
                )

            mms = [0, 0]

            def do_chunk(cat32, w, pb, par):
                """cat32: [128, 2*w] int32 = [yp vals | y vals], w % 6 == 0."""
                ns_c = ns + par
                acc = accs[par]
                nblk = 2 * w // G
                half = w // G                       # yp blocks per half
                pl3 = pb.rearrange("p (b f) -> p b f", f=BLK)[:, :nblk]
                cat16 = cat_pool.tile([P, 2 * fp], bf16, tag="cat16")
                c16 = cat16[:, : 2 * w]
                nc.scalar.activation(c16[:], cat32[:], Copy)
                c16_3 = c16[:].rearrange("p (b f) -> p b f", f=G)
                c32_3 = cat32[:].rearrange("p (b f) -> p b f", f=G)
                for s in range(ns_c):
                    nc.scalar.activation(
                        pl3[:, :, s * G : (s + 1) * G],
                        c32_3[:],
                        Sign,
                        bias=sbias[:, s : s + 1],
                    )
                for s in range(ns_c, C - 1):
                    nc.vector.tensor_scalar(
                        pl3[:, :, s * G : (s + 1) * G],
                        c16_3[:],
                        float(s),
                        None,
                        is_equal,
                    )
                for t in range(half):
                    nc.tensor.matmul(
                        acc[:, :],
                        pb[:, t * BLK : (t + 1) * BLK],
                        pb[:, (half + t) * BLK : (half + t) * BLK + M],
                        start=(mms[par] == 0),
                        stop=(mms[par] == mms_total[par] - 1),
                        skip_group_check=True,
                    )
                    mms[par] += 1

            off = 0
            for wi, w in enumerate(sched):
                is_last = wi == len(sched) - 1
                is_pad = is_last and tail_pad > 0
                real = w - tail_pad if is_pad else w
                ct = cat_pool.tile([P, 2 * fp], i32, tag="cat32", bufs=5)
                cw = ct[:, : 2 * w]
                if is_pad:
                    nc.vector.memset(cw[:], SENTINEL)
                nc.sync.dma_start(cw[:, :real], yp[:, off : off + real])
                nc.sync.dma_start(cw[:, w : w + real], y[:, off : off + real])
                if is_last and planes_tail is not None and w <= RAMP:
                    pb = planes_tail
                else:
                    pb = planes_bufs[wi % 2]
                do_chunk(cw, w, pb, parity[wi])
                off += real
            assert off == n_free

            assert mms == mms_total, (mms, mms_total)
            res = singles.tile([M, 2 * M], f32)
            nc.vector.tensor_copy(res[:, :M], accs[0][:M, :])
            nc.vector.tensor_copy(res[:, M:], accs[1][:M, :])
            nc.sync.dma_start(out, res[:])

    nc.compile()
    return nc


def _get(n_free):
    if n_free not in _CACHE:
        _CACHE[n_free] = _build(n_free)
    return _CACHE[n_free]


def kernel(yp, y, res, n_classes, _trace=False):
    from concourse import bass_utils

    yp = np.ascontiguousarray(np.asarray(yp))
    y = np.ascontiguousarray(np.asarray(y))
    B = yp.shape[0]
    n_free = yp[0].size // P
    nc = _get(n_free)
    in_maps = [
        {"yp": yp[b].reshape(P, n_free), "y": y[b].reshape(P, n_free)}
        for b in range(B)
    ]
    r = bass_utils.run_bass_kernel_spmd(
        nc, in_maps, core_ids=list(range(B)), trace=_trace
    )

    FA, uA = _f_matrix(NS)
    FB, _ = _f_matrix(NS + 1)
    _, rem = _schedule(n_free, FP)
    # sentinel pads live in the final chunk, which has parity A (basis NS)
    npad = ((-rem % G) * P) if rem else 0
    pad_contrib = npad * np.outer(uA, uA)

    def _dec(F, X):
        return np.linalg.solve(F, np.linalg.solve(F, X.T).T)

    outs = []
    for b in range(B):
        Pm = r.results[b]["out"].astype(np.float64)
        XA = np.zeros((C, C), np.float64)
        XB = np.zeros((C, C), np.float64)
        for g in range(G):
            XA += Pm[g::G, g::G][:, : C]
            XB += Pm[g::G, M + g :: G]
        XA -= pad_contrib
        Cnt = _dec(FA, XA) + _dec(FB, XB)
        outs.append(np.round(Cnt))
    res_np = np.stack(outs).astype(np.float32)
    if _trace:
        kernel._last_results = r
    return res_np


# revision 26
# speedup vs baseline: 1.0022x; 1.0022x over previous
"""Confusion-matrix (joint histogram) kernel for Trainium2.

Math: out[b, i, j] = #{pixels p in batch b : yp[b,p] == i and y[b,p] == j}
for i, j in [0, 21). Inputs yp, y are [8, 2048, 2048] int32, values in [0, 21).

Per NeuronCore (core c processes batch c), mixed-basis encoding:
each class-slot column s of a 128-wide block holds f_s(v) where
  slots 0..NS-1   : sign-range masks  S_{s+1}(v) = sign(v - s - 0.5)  (ACT,
                    reads int32 directly, one pass per slot)
  slots NS..19    : one-hot masks     [v == s]                        (DVE
                    is_equal at 4x mode; optionally a few on GpSimd)
  slot 20         : constant 1.0  (memset once per plane buffer)
  cols 126..127   : padding so weight blocks are 128 wide (enables the
                    compiler's fast-weight-load path); contents garbage,
                    confined to out rows 126/127 which the host ignores.

TensorE accumulates X' = F C F^T over all pixel blocks (G=6 pixel-column
groups per 128-wide block, moving operand 126 cols), where C is the true
confusion matrix and F the slot-encoding matrix. Host decodes
C = F^-1 X F^-T exactly in float64 (all X entries are integers < 2^24,
sign masks are +-1 so PSUM fp32 accumulation is exact).
"""

import numpy as np

C = 21                  # classes
G = 6                   # pixel-column groups per block
M = G * C               # 126 used columns
BLK = 128               # padded block width (weights FWL wants 128)
P = 128                 # partitions
FP = 840                # pixel-chunk columns per tensor (divisible by 6)
NS = 4                  # sign-mask slots computed on ACT (slots 0..NS-1)
N_GP = 0                # trailing delta slots computed on GpSimd
ONES_SLOT = C - 1       # slot 20: constant ones (marginals)
SENTINEL = 64           # int32 pad value outside [0, 21)
N_FREE = 32768          # 2048*2048 / 128

_CACHE = {}

RAMP = 216              # small first/last chunk: shrinks PE ramp-in + drain


def _schedule(n_free, fp=FP):
    """Chunk widths (all %6==0) plus trailing real-column remainder for a
    small final chunk (padded to %6 with sentinels by the builder)."""
    rem = 210 + (n_free % 6)
    if n_free <= rem + RAMP + fp:
        rem = n_free % 6
        widths = [n_free - rem] if n_free - rem else []
        return widths, rem
    body = n_free - rem
    k = (body - RAMP) // fp
    leftover = body - RAMP - k * fp
    widths = [RAMP] + [fp] * k
    if leftover:
        widths.append(leftover)
    return widths, rem


def _f_matrix(ns=NS):
    """F[s, v] = f_s(v): slot-encoding matrix, and its value at SENTINEL."""
    F = np.zeros((C, C), dtype=np.float64)
    v = np.arange(C)
    for s in range(ns):
        F[s] = np.where(v >= s + 1, 1.0, -1.0)
    for s in range(ns, C - 1):
        F[s, s] = 1.0
    F[C - 1] = 1.0
    u = np.zeros(C, dtype=np.float64)
    u[:ns] = 1.0          # sign(SENTINEL - s - 0.5) = +1
    u[C - 1] = 1.0        # ones
    assert abs(np.linalg.det(F)) > 0.5
    return F, u


def _build(n_free=N_FREE, fp=FP, ns=NS, n_gp=N_GP):
    import concourse.bacc as bacc
    import concourse.mybir as mybir
    import concourse.tile as tile

    nc = bacc.Bacc(
        "TRN2",
        target_bir_lowering=False,
        debug=False,
        enable_asserts=False,
        num_devices=8,
    )
    yp = nc.dram_tensor("yp", [P, n_free], mybir.dt.int32, kind="ExternalInput").ap()
    y = nc.dram_tensor("y", [P, n_free], mybir.dt.int32, kind="ExternalInput").ap()
    out = nc.dram_tensor("out", [M, M], mybir.dt.float32, kind="ExternalOutput").ap()

    sched, rem = _schedule(n_free, fp)
    tail_pad = -rem % G if rem else 0
    tail_w = rem + tail_pad                      # final (padded) chunk width
    if tail_w:
        sched = sched + [tail_w]
    total_mms = sum(w // G for w in sched)
    nblk_max = 2 * fp // G                       # blocks in a full chunk (both halves)

    bf16 = mybir.dt.bfloat16
    f32 = mybir.dt.float32
    i32 = mybir.dt.int32
    Copy = mybir.ActivationFunctionType.Copy
    Sign = mybir.ActivationFunctionType.Sign
    is_equal = mybir.AluOpType.is_equal

    with tile.TileContext(nc) as tc:
        with (
            tc.tile_pool(name="psum", bufs=1, space="PSUM") as psum_pool,
            tc.tile_pool(name="cat", bufs=3) as cat_pool,
            tc.tile_pool(name="singles", bufs=1) as singles,
        ):
            acc = psum_pool.tile([P, M], f32)
            # Per-slot bias vectors for the ACT sign masks (bias must be a
            # [128, 1] SBUF AP for non-Copy activations).
            sbias = singles.tile([P, max(ns, 1)], f32)
            for s in range(ns):
                nc.vector.memset(sbias[:, s : s + 1], -(s + 0.5))
            # Dummy activation: triggers the lazy ACT_TABLE_LOAD (~2.7us)
            # during the first DMA wait instead of on chunk 0's conversion.
            warm = singles.tile([P, 1], bf16)
            nc.scalar.activation(warm[:], sbias[:, 0:1], Sign, bias=sbias[:, 0:1])
            # Two persistent plane buffers (manual double-buffer) so the
            # ones-columns survive across chunks after a single memset.
            planes_bufs = []
            for bi in range(2):
                pb = singles.tile(
                    [P, nblk_max * BLK], bf16, tag=f"planes{bi}", name=f"planes{bi}"
                )
                pv = pb.rearrange("p (b f) -> p b f", f=BLK)
                nc.vector.memset(
                    pv[:, :, ONES_SLOT * G : (ONES_SLOT + 1) * G], 1.0
                )
                planes_bufs.append(pb)
            # Third (small) buffer for the final chunk: frees it from the
            # 2-buffer WAR wait on PE at the end of the stream.
            planes_tail = None
            if 0 < tail_w <= RAMP:
                planes_tail = singles.tile(
                    [P, (2 * tail_w // G) * BLK], bf16, tag="planes_t"
                )
                ptv = planes_tail.rearrange("p (b f) -> p b f", f=BLK)
                nc.vector.memset(
                    ptv[:, :, ONES_SLOT * G : (ONES_SLOT + 1) * G], 1.0
                )

            mm = 0

            def do_chunk(cat32, w, pb):
                """cat32: [128, 2*w] int32 = [yp vals | y vals], w % 6 == 0."""
                nonlocal mm
                nblk = 2 * w // G
                half = w // G                       # yp blocks per half
                pl3 = pb.rearrange("p (b f) -> p b f", f=BLK)[:, :nblk]
                cat16 = cat_pool.tile([P, 2 * fp], bf16, tag="cat16")
                c16 = cat16[:, : 2 * w]
                nc.scalar.activation(c16[:], cat32[:], Copy)
                c16_3 = c16[:].rearrange("p (b f) -> p b f", f=G)
                c32_3 = cat32[:].rearrange("p (b f) -> p b f", f=G)
                for s in range(ns):
                    nc.scalar.activation(
                        pl3[:, :, s * G : (s + 1) * G],
                        c32_3[:],
                        Sign,
                        bias=sbias[:, s : s + 1],
                    )
                n_dve_end = C - 1 - n_gp
                for s in range(ns, n_dve_end):
                    nc.vector.tensor_scalar(
                        pl3[:, :, s * G : (s + 1) * G],
                        c16_3[:],
                        float(s),
                        None,
                        is_equal,
                    )
                for s in range(n_dve_end, C - 1):
                    nc.gpsimd.tensor_scalar(
                        pl3[:, :, s * G : (s + 1) * G],
                        c16_3[:],
                        float(s),
                        None,
                        is_equal,
                    )
                for t in range(half):
                    nc.tensor.matmul(
                        acc[:, :],
                        pb[:, t * BLK : (t + 1) * BLK],
                        pb[:, (half + t) * BLK : (half + t) * BLK + M],
                        start=(mm == 0),
                        stop=(mm == total_mms - 1),
                    )
                    mm += 1

            off = 0
            for wi, w in enumerate(sched):
                is_last = wi == len(sched) - 1
                is_pad = is_last and tail_pad > 0
                real = w - tail_pad if is_pad else w
                ct = cat_pool.tile([P, 2 * fp], i32, tag="cat32", bufs=6)
                cw = ct[:, : 2 * w]
                if is_pad:
                    nc.vector.memset(cw[:], SENTINEL)
                nc.sync.dma_start(cw[:, :real], yp[:, off : off + real])
                nc.sync.dma_start(cw[:, w : w + real], y[:, off : off + real])
                if is_last and planes_tail is not None and w <= RAMP:
                    pb = planes_tail
                else:
                    pb = planes_bufs[wi % 2]
                do_chunk(cw, w, pb)
                off += real
            assert off == n_free

            assert mm == total_mms
            res = singles.tile([M, M], f32)
            nc.vector.tensor_copy(res[:], acc[:M, :])
            nc.sync.dma_start(out, res[:])

    nc.compile()
    return nc


def _get(n_free):
    if n_free not in _CACHE:
        _CACHE[n_free] = _build(n_free)
    return _CACHE[n_free]


def kernel(yp, y, res, n_classes, _trace=False):
    from concourse import bass_utils

    yp = np.ascontiguousarray(np.asarray(yp))
    y = np.ascontiguousarray(np.asarray(y))
    B = yp.shape[0]
    n_free = yp[0].size // P
    nc = _get(n_free)
    in_maps = [
        {"yp": yp[b].reshape(P, n_free), "y": y[b].reshape(P, n_free)}
        for b in range(B)
    ]
    r = bass_utils.run_bass_kernel_spmd(
        nc, in_maps, core_ids=list(range(B)), trace=_trace
    )

    F, u = _f_matrix()
    _, rem = _schedule(n_free, FP)
    npad = ((-rem % G) * P) if rem else 0
    pad_contrib = npad * np.outer(u, u)

    outs = []
    for b in range(B):
        Pm = r.results[b]["out"].astype(np.float64)
        X = np.zeros((C, C), np.float64)
        for g in range(G):
            X += Pm[g::G, g::G]
        X -= pad_contrib
        Cnt = np.linalg.solve(F, np.linalg.solve(F, X.T).T)
        outs.append(np.round(Cnt))
    res_np = np.stack(outs).astype(np.float32)
    if _trace:
        kernel._last_results = r
    return res_np
